# revision 1
# baseline (speedup 1.0000x reference)
"""Multi-head attention (B=2, N=2048, D=1024, H=16, RoPE, dense softmax) on
8 Trainium2 NeuronCores.

Sharding: data-parallel over batch (cores 0-3 -> b=0, 4-7 -> b=1), tensor-
parallel over heads (each core owns 4 of the 16 heads, i.e. 256 of the 1024
hidden dims of Wq/Wk/Wv rows and Wo columns). Each core computes its heads'
attention and a partial output projection; the host sums the 4 partials per
batch.

Device layout notes:
  - All matmul operands are float16 (PE runs 1 cycle/row with fast weight
    load; fp32 and float32r pay a serialized ~218ns LDWEIGHTS per matmul).
    PSUM accumulation and all elementwise math stay fp32.
  - x is fed pre-transposed (xT [D, N]) so the D contraction sits on the
    partition dim; Q^T/K^T are produced head-pair-packed [128, N] and RoPE'd
    in place; V is produced in [keys, head*64] layout with an extra ones
    column so the P@V matmul also yields the softmax denominators.
  - Scores are computed as S^T [keys, q] in double-buffered 2-bank PSUM
    tiles (one per key chunk x query-tile pair) so QK^T of chunk k+1
    overlaps the Exp of chunk k and the PE stays continuously busy; the
    attention mask is ignored (it is all-ones for this problem).
"""

import os
import numpy as np

import concourse.bass as bass
from concourse import bacc
import concourse.mybir as mybir
import concourse.tile as tile
from concourse.bass_utils import run_bass_kernel_spmd

dt = mybir.dt

B, N, D, H, HD = 2, 2048, 1024, 16, 64
NCORES = 8
HPC = H * B // NCORES          # 4 heads per core
DPC = HPC * HD                 # 256 owned hidden dims per core
QT = 512                       # query tile (free dim of QK^T / PV matmuls)
NQT = N // QT                  # 4 query tiles
KC = 128                       # key chunk (partition dim of S^T)
NKC = N // KC                  # 16 key chunks
KG = 4                         # key chunks per exp group (4 PSUM banks)
DC = D // 128                  # 8 contraction chunks for projections
SCALE = float(HD) ** -0.5

MMDT = dt.float16          # matmul operand dtype (PE: 1 cyc/row + FWL)
NPMM = np.float16
F32 = dt.float32


def build_nc():
    nc = bacc.Bacc("TRN2")
    xT = nc.dram_tensor("xT", [D, N], MMDT, kind="ExternalInput")
    wqT = nc.dram_tensor("wqT", [D, DPC], MMDT, kind="ExternalInput")
    wkT = nc.dram_tensor("wkT", [D, DPC], MMDT, kind="ExternalInput")
    wvT = nc.dram_tensor("wvT", [D, DPC], MMDT, kind="ExternalInput")
    woT = nc.dram_tensor("woT", [DPC, D], MMDT, kind="ExternalInput")
    cosT = nc.dram_tensor("cosT", [128, N], F32, kind="ExternalInput")
    msinT = nc.dram_tensor("msinT", [128, N], F32, kind="ExternalInput")
    vones = nc.dram_tensor("vones", [128, NKC, HPC, 1], MMDT, kind="ExternalInput")
    out = nc.dram_tensor("out", [N, D], F32, kind="ExternalOutput")

    with tile.TileContext(nc) as tc:
        with tc.tile_pool(name="big", bufs=8) as big, \
             tc.tile_pool(name="persist", bufs=1) as persist, \
             tc.tile_pool(name="scratch", bufs=3) as scratch, \
             tc.tile_pool(name="outp", bufs=4) as outp, \
             tc.tile_pool(name="ps_st", bufs=2, space="PSUM") as ps_st, \
             tc.tile_pool(name="ps_any", bufs=4, space="PSUM") as ps_any:

            # ---- persistent SBUF tensors ----
            x_s = []
            for d in range(DC):
                xt = big.tile([128, N], MMDT, name=f"x_s{d}", tag="big")
                nc.sync.dma_start(out=xt, in_=xT[d * 128:(d + 1) * 128, :])
                x_s.append(xt)

            wq_s = persist.tile([128, DC, DPC], MMDT, name="wq_s")
            wk_s = persist.tile([128, DC, DPC], MMDT, name="wk_s")
            wv_s = persist.tile([128, DC, DPC], MMDT, name="wv_s")
            nc.sync.dma_start(out=wq_s, in_=wqT.rearrange("(d p) c -> p d c", p=128))
            nc.sync.dma_start(out=wk_s, in_=wkT.rearrange("(d p) c -> p d c", p=128))
            nc.sync.dma_start(out=wv_s, in_=wvT.rearrange("(d p) c -> p d c", p=128))
            wo_s = persist.tile([128, DPC // 128, D], MMDT, name="wo_s")
            nc.sync.dma_start(out=wo_s, in_=woT.rearrange("(d p) c -> p d c", p=128))

            cos_s = persist.tile([128, N], F32, name="cos_s")
            msin_s = persist.tile([128, N], F32, name="msin_s")
            nc.sync.dma_start(out=cos_s, in_=cosT[:, :])
            nc.sync.dma_start(out=msin_s, in_=msinT[:, :])

            qT_s = persist.tile([128, 2, N], MMDT, name="qT_s")
            kT_s = persist.tile([128, 2, N], MMDT, name="kT_s")
            # V with ones column: [keys(128), kchunk, head, 65]
            v_s = persist.tile([128, NKC, HPC, HD + 1], MMDT, name="v_s")
            nc.gpsimd.dma_start(out=v_s[:, :, :, HD:HD + 1], in_=vones[:, :, :, :])
            attnT_s = persist.tile([128, 2, N], MMDT, name="attnT_s")

            # ---- phase 1: projections + RoPE ----
            def rope(dst, psum, tq):
                cs = cos_s[:, tq * QT:(tq + 1) * QT]
                ms = msin_s[:, tq * QT:(tq + 1) * QT]
                nc.vector.tensor_mul(out=dst, in0=psum, in1=cs)
                t2 = scratch.tile([128, QT], F32, name="t2", tag="t2")
                for r in (0, 32, 64, 96):
                    pr = r ^ 32
                    nc.vector.tensor_mul(out=t2[r:r + 32, :],
                                         in0=psum[pr:pr + 32, :],
                                         in1=ms[r:r + 32, :])
                nc.vector.tensor_add(out=dst, in0=dst, in1=t2)

            # Projections, ordered so the attention phase's prerequisites
            # (K^T pair 0, all of V, Q^T pair 0) finish earliest -- pair-1
            # projections then overlap the start of attention.
            def project_qk(w_s, dstT, i):
                for t2 in range(NQT // 2):
                    ps = [ps_any.tile([128, QT], F32, name=f"pp{u}",
                                      tag="any") for u in range(2)]
                    for d in range(DC):
                        wsl = w_s[:, d, i * 128:(i + 1) * 128]
                        for u in range(2):
                            t = t2 * 2 + u
                            nc.tensor.matmul(
                                ps[u], wsl,
                                x_s[d][:, t * QT:(t + 1) * QT],
                                start=(d == 0), stop=(d == DC - 1))
                    for u in range(2):
                        t = t2 * 2 + u
                        rope(dstT[:, i, t * QT:(t + 1) * QT], ps[u], t)

            def project_v():
                for k in range(NKC):
                    pv = ps_any.tile([128, DPC], F32, name="pv", tag="any")
                    for d in range(DC):
                        nc.tensor.matmul(pv,
                                         x_s[d][:, k * KC:(k + 1) * KC],
                                         wv_s[:, d, :],
                                         start=(d == 0), stop=(d == DC - 1))
                    for h in range(HPC):
                        nc.vector.tensor_copy(out=v_s[:, k, h, 0:HD],
                                              in_=pv[:, h * HD:(h + 1) * HD])

            project_qk(wk_s, kT_s, 0)
            project_v()
            project_qk(wq_s, qT_s, 0)
            project_qk(wk_s, kT_s, 1)
            project_qk(wq_s, qT_s, 1)

            # ---- phase 2: attention + output projection ----
            # Query tiles are processed in PAIRS so each stationary operand
            # (K^T chunk for QK^T, V_aug chunk for PV) is loaded into the PE
            # array once per two matmuls, halving LDWEIGHTS traffic.
            for t2 in range(NQT // 2):
                for i in range(2):
                    for hl in range(2):
                        h = i * 2 + hl
                        r0 = hl * HD
                        accs = [ps_any.tile([HD + 1, QT], F32,
                                           name=f"acc{u}", tag="any")
                                for u in range(2)]
                        qsl = [qT_s[r0:r0 + HD, i,
                                    (t2 * 2 + u) * QT:(t2 * 2 + u + 1) * QT]
                               for u in range(2)]
                        for k in range(NKC):
                            # [128, 2, QT] score tile (2 banks), double-
                            # buffered so QK of chunk k+1 overlaps exp(k):
                            # keeps the PE continuously busy (HAM warm).
                            st = ps_st.tile([128, 2, QT], F32, name="st",
                                            tag="st")
                            ksl = kT_s[r0:r0 + HD, i, k * KC:(k + 1) * KC]
                            for u in range(2):
                                nc.tensor.matmul(st[:, u, :], ksl, qsl[u],
                                                 start=True, stop=True)
                            pt = big.tile([128, 2 * QT], MMDT, name="pt",
                                          tag="big")
                            nc.scalar.activation(
                                out=pt, in_=st.rearrange("p a b -> p (a b)"),
                                func=mybir.ActivationFunctionType.Exp,
                                scale=SCALE)
                            vsl = v_s[:, k, h, :]
                            for u in range(2):
                                nc.tensor.matmul(
                                    accs[u], vsl,
                                    pt[:, u * QT:(u + 1) * QT],
                                    start=(k == 0), stop=(k == NKC - 1),
                                    skip_group_check=True)
                        # normalize: approx-reciprocal of the denominator row,
                        # GPSIMD partition-broadcast to 64 rows (SBUF), then a
                        # single fused psum*sbuf multiply into attnT.
                        prow = (h % 2) * HD
                        slot = h // 2
                        for u in range(2):
                            t = t2 * 2 + u
                            # custom-DVE ops misread PSUM at partition offset
                            # 64 on HW; stage the row through SBUF first.
                            den_raw = scratch.tile([1, QT], F32,
                                                   name="den_raw", tag="denr")
                            nc.vector.tensor_copy(out=den_raw,
                                                  in_=accs[u][HD:HD + 1, :])
                            den = scratch.tile([1, QT], F32, name="den",
                                               tag="den")
                            nc.vector.reciprocal_approx_fast(
                                out=den, in_=den_raw)
                            bca = scratch.tile([HD, QT], F32, name="bca",
                                               tag="bca")
                            nc.gpsimd.partition_broadcast(bca, den)
                            nc.vector.tensor_mul(
                                out=attnT_s[prow:prow + HD, slot,
                                            t * QT:(t + 1) * QT],
                                in0=accs[u][0:HD, :], in1=bca)

                # output projection for this query-tile pair; dc outer / e
                # inner so the attnT stationary is shared by 2 matmuls.
                for qc in range(2 * QT // 128):
                    q0 = t2 * 2 * QT + qc * 128
                    ot = outp.tile([128, D], F32, name="ot", tag="out")
                    pos = [ps_any.tile([128, 512], F32, name=f"po{e}",
                                      tag="any") for e in range(2)]
                    for dc in range(DPC // 128):
                        asl = attnT_s[:, dc, q0:q0 + 128]
                        for e in range(2):
                            nc.tensor.matmul(
                                pos[e], asl,
                                wo_s[:, dc, e * 512:(e + 1) * 512],
                                start=(dc == 0), stop=(dc == DPC // 128 - 1))
                    for e in range(2):
                        nc.vector.tensor_copy(out=ot[:, e * 512:(e + 1) * 512],
                                              in_=pos[e])
                    nc.gpsimd.dma_start(out=out[q0:q0 + 128, :], in_=ot)
    nc.finalize()
    return nc


_NC_CACHE = None


def _get_nc():
    global _NC_CACHE
    if _NC_CACHE is None:
        _NC_CACHE = build_nc()
    return _NC_CACHE


def _rope_tables():
    inv_freq = 1.0 / (10000.0 ** (np.arange(0, HD, 2, dtype=np.float32) / HD))
    t = np.arange(N, dtype=np.float32)
    freqs = np.outer(t, inv_freq).astype(np.float32)       # [N, 32]
    emb = np.concatenate([freqs, freqs], axis=-1)          # [N, 64]
    cos = np.cos(emb).astype(np.float32)                   # [N, 64]
    sin = np.sin(emb).astype(np.float32)
    idx = np.arange(128) % HD
    cosT = np.ascontiguousarray(cos.T[idx])                # [128, N]
    sgn = np.where(np.arange(HD) < HD // 2, -1.0, 1.0).astype(np.float32)
    msinT = np.ascontiguousarray((sin.T * sgn[:, None])[idx])
    return cosT, msinT


def kernel(x, attention_mask, Wq, Wk, Wv, Wo):
    x = np.asarray(x, dtype=np.float32)
    Wq = np.asarray(Wq, dtype=np.float32)
    Wk = np.asarray(Wk, dtype=np.float32)
    Wv = np.asarray(Wv, dtype=np.float32)
    Wo = np.asarray(Wo, dtype=np.float32)

    cosT, msinT = _rope_tables()
    xTb = [np.ascontiguousarray(x[b].T).astype(NPMM) for b in range(B)]

    in_maps = []
    for c in range(NCORES):
        b = c // (NCORES // B)
        hg = c % (NCORES // B)
        rows = slice(hg * DPC, (hg + 1) * DPC)
        in_maps.append({
            "xT": xTb[b],
            "wqT": np.ascontiguousarray(Wq[rows].T).astype(NPMM),
            "wkT": np.ascontiguousarray(Wk[rows].T).astype(NPMM),
            "wvT": np.ascontiguousarray(Wv[rows].T).astype(NPMM),
            "woT": np.ascontiguousarray(Wo[:, rows].T).astype(NPMM),
            "cosT": cosT,
            "msinT": msinT,
            "vones": np.ones((128, NKC, HPC, 1), dtype=NPMM),
        })

    global _last_in_maps
    _last_in_maps = in_maps

    nc = _get_nc()
    res = run_bass_kernel_spmd(nc, in_maps, core_ids=list(range(NCORES)))
    parts = [r["out"] for r in res.results]

    out = np.empty((B, N, D), dtype=np.float32)
    g = NCORES // B
    for b in range(B):
        out[b] = np.sum(np.stack(parts[b * g:(b + 1) * g]), axis=0)
    return out



# revision 23
# speedup vs baseline: 1.0149x; 1.0149x over previous
"""Multi-head attention (B=2, N=2048, D=1024, H=16, RoPE, dense softmax) on
8 Trainium2 NeuronCores.

Sharding: data-parallel over batch (cores 0-3 -> b=0, 4-7 -> b=1), tensor-
parallel over heads (each core owns 4 of the 16 heads, i.e. 256 of the 1024
hidden dims of Wq/Wk/Wv rows and Wo columns). Each core computes its heads'
attention and a partial output projection; the host sums the 4 partials per
batch.

Single merged pipeline: the attention chunk loop starts as soon as K/Q for
head-pair 0 and the first V chunks exist; all remaining projection work
(RoPE'd Q/K tiles, V chunks) plus the output projection is chopped into
small "filler" thunks that are drained between attention chunk iterations,
keeping the PE and DVE busy underneath the ACT-bound exp stream.

Key device-level choices:
  - matmul operands fp16 (1 cyc/row + fast weight load); PSUM fp32.
  - scores computed as S^T [keys, q] in double-buffered 2-bank PSUM tiles;
    exp on ScalarE in [128,1024] tiles (the pipeline's rate limiter).
  - V carries a leading ones column so the P@V matmul also emits the softmax
    denominators in PSUM partition 0 (readable by the custom fast-reciprocal,
    which cannot read PSUM at partition offset 64).
  - per-iteration issue order is QK(next chunk) -> exp -> fillers -> PV so a
    lagging filler or PV never starves the ScalarE exp stream.
  - weights/x/tables are host-prepacked into the exact SBUF layouts so every
    input DMA is a contiguous [128, 1024+] transfer.
"""

import collections
import os
import numpy as np

import concourse.bass as bass
from concourse import bacc
import concourse.mybir as mybir
import concourse.tile as tile
from concourse.bass_utils import run_bass_kernel_spmd

dt = mybir.dt

B, N, D, H, HD = 2, 2048, 1024, 16, 64
NCORES = 8
HPC = H * B // NCORES          # 4 heads per core
DPC = HPC * HD                 # 256 owned hidden dims per core
QT = 512                       # query tile (free dim of QK^T / PV matmuls)
NQT = N // QT                  # 4 query tiles
KC = 128                       # key chunk (partition dim of S^T)
NKC = N // KC                  # 16 key chunks
DC = D // 128                  # 8 contraction chunks for projections
SCALE = float(HD) ** -0.5

F16 = dt.float16
F32 = dt.float32
NP16 = np.float16


def build_nc():
    nc = bacc.Bacc("TRN2")
    xT = nc.dram_tensor("xT", [D, N], F16, kind="ExternalInput")
    wqd = nc.dram_tensor("wqd", [128, DC * DPC], F16, kind="ExternalInput")
    wkd = nc.dram_tensor("wkd", [128, DC * DPC], F16, kind="ExternalInput")
    wvd = nc.dram_tensor("wvd", [128, DC * DPC], F16, kind="ExternalInput")
    wod = nc.dram_tensor("wod", [128, 2 * D], F16, kind="ExternalInput")
    cosd = nc.dram_tensor("cosd", [128, N], F16, kind="ExternalInput")
    # msin with partition rows pre-permuted by r^32 so the RoPE rotate-half
    # multiply reads both SBUF operands from the same base partition.
    msinPd = nc.dram_tensor("msinPd", [128, N], F16, kind="ExternalInput")
    out = nc.dram_tensor("out", [N, D], F16, kind="ExternalOutput")
    dbg = os.environ.get("KDEBUG")
    if dbg:
        dbg_qT = nc.dram_tensor("dbg_qT", [128, 2, N], F16, kind="ExternalOutput")
        dbg_kT = nc.dram_tensor("dbg_kT", [128, 2, N], F16, kind="ExternalOutput")
        dbg_v = nc.dram_tensor("dbg_v", [128, NKC, HPC, HD + 1], F16, kind="ExternalOutput")
        dbg_attnT = nc.dram_tensor("dbg_attnT", [128, 2, N], F16, kind="ExternalOutput")

    with tile.TileContext(nc) as tc:
        with tc.tile_pool(name="xp", bufs=1) as xp, \
             tc.tile_pool(name="persist", bufs=1) as persist, \
             tc.tile_pool(name="ptp", bufs=3) as ptp, \
             tc.tile_pool(name="ropep", bufs=2) as ropep, \
             tc.tile_pool(name="normp", bufs=2) as normp, \
             tc.tile_pool(name="outp", bufs=4) as outp, \
             tc.tile_pool(name="ps_st", bufs=2, space="PSUM") as ps_st, \
             tc.tile_pool(name="ps_acc", bufs=1, space="PSUM") as ps_acc, \
             tc.tile_pool(name="ps_misc", bufs=2, space="PSUM") as ps_misc:

            # ---- persistent SBUF tensors; DMAs in dependency-priority order
            wv_s = persist.tile([128, DC, DPC], F16, name="wv_s")
            wk_s = persist.tile([128, DC, DPC], F16, name="wk_s")
            wq_s = persist.tile([128, DC, DPC], F16, name="wq_s")
            nc.sync.dma_start(out=wv_s, in_=wvd[:, :])
            nc.sync.dma_start(out=wk_s, in_=wkd[:, :])
            nc.sync.dma_start(out=wq_s, in_=wqd[:, :])
            x_s = []
            for d in range(DC):
                xt = xp.tile([128, N], F16, name=f"x_s{d}", tag="x", bufs=DC)
                nc.sync.dma_start(out=xt[:, 0:1024],
                                  in_=xT[d * 128:(d + 1) * 128, 0:1024])
                x_s.append(xt)
            cos_s = persist.tile([128, N], F16, name="cos_s")
            msinP_s = persist.tile([128, N], F16, name="msinP_s")
            nc.sync.dma_start(out=cos_s, in_=cosd[:, :])
            nc.sync.dma_start(out=msinP_s, in_=msinPd[:, :])
            for d in range(DC):
                nc.sync.dma_start(out=x_s[d][:, 1024:2048],
                                  in_=xT[d * 128:(d + 1) * 128, 1024:2048])
            wo_s = persist.tile([128, 2, D], F16, name="wo_s")
            nc.sync.dma_start(out=wo_s, in_=wod[:, :])

            qT_s = persist.tile([128, 2, N], F16, name="qT_s")
            kT_s = persist.tile([128, 2, N], F16, name="kT_s")
            # V with trailing ones column: [keys(128), kchunk, head, 64+1]
            # (ones last, not first: the PV output's O^T rows must start at
            # partition 0 — partition offsets have 32-alignment rules)
            v_s = persist.tile([128, NKC, HPC, HD + 1], F16, name="v_s")
            nc.gpsimd.memset(v_s[:, :, :, HD:HD + 1], 1.0)
            attnT_s = persist.tile([128, 2, N], F16, name="attnT_s")

            # ---- projection / filler units (generators yielding ~400ns of
            # PE work per step so they can be drained between chunk iters)
            def gen_proj(w_s, dstT, i, t):
                """Project one [128, QT] q/k tile for head-pair i and RoPE it."""
                ps = ps_misc.tile([128, QT], F32, name="pps", tag="m")
                for d2 in range(DC // 2):
                    for d in (2 * d2, 2 * d2 + 1):
                        nc.tensor.matmul(
                            ps, w_s[:, d, i * 128:(i + 1) * 128],
                            x_s[d][:, t * QT:(t + 1) * QT],
                            start=(d == 0), stop=(d == DC - 1))
                    yield
                # stage PSUM->SBUF fp16 (frees the misc bank fast), then RoPE
                # entirely in fp16 at 2x DVE rate.
                qf = ropep.tile([128, QT], F16, name="qf", tag="qf")
                nc.vector.tensor_copy(out=qf, in_=ps)
                cs = cos_s[:, t * QT:(t + 1) * QT]
                ms = msinP_s[:, t * QT:(t + 1) * QT]
                tf = ropep.tile([128, QT], F16, name="tf", tag="tf")
                for r in (0, 32, 64, 96):
                    pr = r ^ 32
                    nc.vector.tensor_mul(out=tf[r:r + 32, :],
                                         in0=qf[pr:pr + 32, :],
                                         in1=ms[pr:pr + 32, :])
                dst = dstT[:, i, t * QT:(t + 1) * QT]
                nc.vector.tensor_mul(out=dst, in0=qf, in1=cs)
                nc.vector.tensor_add(out=dst, in0=dst, in1=tf)
                yield

            def gen_v(k):
                """Project V chunk k (all 4 heads) into v_s[:, k, :, 1:]."""
                pv = ps_misc.tile([128, DPC], F32, name="pvv", tag="m")
                for d2 in range(DC // 2):
                    for d in (2 * d2, 2 * d2 + 1):
                        nc.tensor.matmul(pv,
                                         x_s[d][:, k * KC:(k + 1) * KC],
                                         wv_s[:, d, :],
                                         start=(d == 0), stop=(d == DC - 1))
                    yield
                nc.vector.tensor_copy(
                    out=v_s[:, k, :, 0:HD],
                    in_=pv.rearrange("p (h e) -> p h e", h=HPC))
                yield

            def gen_outproj(t2q, qc):
                """Output projection for one 128-query chunk of tile-pair t2q."""
                q0 = t2q * 2 * QT + qc * 128
                ot = outp.tile([128, D], F16, name="ot", tag="ot")
                for e in range(2):
                    pos = ps_misc.tile([128, 512], F32, name="pos", tag="m")
                    for dc in range(2):
                        nc.tensor.matmul(
                            pos, attnT_s[:, dc, q0:q0 + 128],
                            wo_s[:, dc, e * 512:(e + 1) * 512],
                            start=(dc == 0), stop=(dc == 1))
                    yield
                    nc.vector.tensor_copy(out=ot[:, e * 512:(e + 1) * 512],
                                          in_=pos)
                nc.gpsimd.dma_start(out=out[q0:q0 + 128, :], in_=ot)
                yield

            filler = collections.deque()
            done = set()

            def tracked(g, key):
                yield from g
                done.add(key)

            def drain(n):
                for _ in range(n):
                    while filler:
                        try:
                            next(filler[0])
                            break
                        except StopIteration:
                            filler.popleft()
                    else:
                        break

            def force(key):
                # Correctness guard: a consumer must never be ISSUED before
                # its producer (Tile tracks deps in issue order), so drain
                # the filler queue until the producer unit has been emitted.
                while key not in done:
                    assert filler, f"filler ran dry before {key}"
                    drain(1)

            def run_unit(g, key):
                for _ in g:
                    pass
                done.add(key)

            # ---- prologue: just enough for attention block 0 to start
            run_unit(gen_proj(wk_s, kT_s, 0, 0), ("k", 0, 0))
            run_unit(gen_proj(wq_s, qT_s, 0, 0), ("q", 0, 0))
            run_unit(gen_proj(wq_s, qT_s, 0, 1), ("q", 0, 1))
            run_unit(gen_proj(wk_s, kT_s, 0, 1), ("k", 0, 1))
            run_unit(gen_proj(wk_s, kT_s, 0, 2), ("k", 0, 2))
            run_unit(gen_v(0), ("v", 0))
            run_unit(gen_v(1), ("v", 1))

            units = [
                (("v", 2), gen_v(2)), (("v", 3), gen_v(3)),
                (("k", 0, 3), gen_proj(wk_s, kT_s, 0, 3)),
                (("v", 4), gen_v(4)), (("v", 5), gen_v(5)),
                (("v", 6), gen_v(6)), (("v", 7), gen_v(7)),
                (("v", 8), gen_v(8)), (("v", 9), gen_v(9)),
                (("k", 1, 0), gen_proj(wk_s, kT_s, 1, 0)),
                (("v", 10), gen_v(10)), (("v", 11), gen_v(11)),
                (("q", 1, 0), gen_proj(wq_s, qT_s, 1, 0)),
                (("v", 12), gen_v(12)), (("v", 13), gen_v(13)),
                (("q", 1, 1), gen_proj(wq_s, qT_s, 1, 1)),
                (("v", 14), gen_v(14)), (("v", 15), gen_v(15)),
                (("k", 1, 1), gen_proj(wk_s, kT_s, 1, 1)),
                (("k", 1, 2), gen_proj(wk_s, kT_s, 1, 2)),
                (("k", 1, 3), gen_proj(wk_s, kT_s, 1, 3)),
                (("q", 0, 2), gen_proj(wq_s, qT_s, 0, 2)),
                (("q", 0, 3), gen_proj(wq_s, qT_s, 0, 3)),
                (("q", 1, 2), gen_proj(wq_s, qT_s, 1, 2)),
                (("q", 1, 3), gen_proj(wq_s, qT_s, 1, 3)),
            ]
            for key, g in units:
                filler.append(tracked(g, key))

            # ---- attention: 8 blocks of 16 key-chunk iterations
            # drain rates front-load the projection fillers while the exp
            # pipeline is still filling.
            rates = [3, 3, 2, 2, 1, 1, 1, 1]
            for t2q in range(2):
                for i in range(2):
                    for hl in range(2):
                        bidx = t2q * 4 + i * 2 + hl
                        rate = rates[bidx]
                        h = i * 2 + hl
                        r0 = hl * HD
                        acc = ps_acc.tile([HD + 1, 2, QT], F32, name="acc",
                                          tag="acc")
                        qsl = [qT_s[r0:r0 + HD, i,
                                    (2 * t2q + u) * QT:(2 * t2q + u + 1) * QT]
                               for u in range(2)]
                        def pv(k, pt):
                            force(("v", k))
                            vsl = v_s[:, k, h, :]
                            for u in range(2):
                                nc.tensor.matmul(
                                    acc[:, u, :], vsl, pt[:, u, :],
                                    start=(k == 0), stop=(k == NKC - 1),
                                    skip_group_check=True)

                        force(("q", i, 2 * t2q))
                        force(("q", i, 2 * t2q + 1))
                        prev = None
                        for k in range(NKC):
                            if k % 4 == 0:
                                force(("k", i, k // 4))
                            st = ps_st.tile([128, 2, QT], F32, name="st",
                                            tag="st")
                            ksl = kT_s[r0:r0 + HD, i, k * KC:(k + 1) * KC]
                            for u in range(2):
                                nc.tensor.matmul(st[:, u, :], ksl, qsl[u],
                                                 start=True, stop=True)
                            pt = ptp.tile([128, 2, QT], F16, name="pt",
                                          tag="pt")
                            nc.scalar.activation(
                                out=pt.rearrange("p a b -> p (a b)"),
                                in_=st.rearrange("p a b -> p (a b)"),
                                func=mybir.ActivationFunctionType.Exp,
                                scale=SCALE)
                            drain(rate)
                            if prev is not None:
                                pv(*prev)
                            prev = (k, pt)
                        pv(*prev)
                        # normalize: den sits in PSUM partition 64 (ones col);
                        # stage it to SBUF (custom-DVE reciprocal can't read
                        # PSUM at partition 64), fast-reciprocal, GPSIMD-
                        # broadcast to 64 rows, one multiply into attnT.
                        for u in range(2):
                            den_raw = normp.tile([1, QT], F32, name="den_raw",
                                                 tag="denr")
                            nc.vector.tensor_copy(out=den_raw,
                                                  in_=acc[HD:HD + 1, u, :])
                            den = normp.tile([1, QT], F32, name="den",
                                             tag="den")
                            nc.vector.reciprocal_approx_fast(
                                out=den, in_=den_raw)
                            bca = normp.tile([HD, QT], F32, name="bca",
                                             tag="bca")
                            nc.gpsimd.partition_broadcast(bca, den)
                            t = 2 * t2q + u
                            nc.vector.tensor_mul(
                                out=attnT_s[r0:r0 + HD, i,
                                            t * QT:(t + 1) * QT],
                                in0=acc[0:HD, u, :], in1=bca)
                # outproj for this tile-pair becomes legal after its 4 blocks
                for qc in range(2 * QT // 128):
                    filler.append(gen_outproj(t2q, qc))
            drain(10 ** 9)
            if dbg:
                nc.sync.dma_start(out=dbg_qT[:, :, :], in_=qT_s)
                nc.sync.dma_start(out=dbg_kT[:, :, :], in_=kT_s)
                nc.sync.dma_start(out=dbg_v[:, :, :, :], in_=v_s)
                nc.sync.dma_start(out=dbg_attnT[:, :, :], in_=attnT_s)
    nc.finalize()
    return nc


_NC_CACHE = None


def _get_nc():
    global _NC_CACHE
    if _NC_CACHE is None:
        _NC_CACHE = build_nc()
    return _NC_CACHE


def _rope_tables():
    inv_freq = 1.0 / (10000.0 ** (np.arange(0, HD, 2, dtype=np.float32) / HD))
    t = np.arange(N, dtype=np.float32)
    freqs = np.outer(t, inv_freq).astype(np.float32)       # [N, 32]
    emb = np.concatenate([freqs, freqs], axis=-1)          # [N, 64]
    cos = np.cos(emb).astype(np.float32)                   # [N, 64]
    sin = np.sin(emb).astype(np.float32)
    idx = np.arange(128) % HD
    cosT = np.ascontiguousarray(cos.T[idx])                # [128, N]
    sgn = np.where(np.arange(HD) < HD // 2, -1.0, 1.0).astype(np.float32)
    msinT = np.ascontiguousarray((sin.T * sgn[:, None])[idx])
    msinP = np.ascontiguousarray(msinT[np.arange(128) ^ 32])
    return cosT.astype(NP16), msinP.astype(NP16)


def _pack_w(wT):
    """[n*128, C] row-major -> [128, n*C] with [p, chunk, c] free layout."""
    n = wT.shape[0] // 128
    return np.ascontiguousarray(
        wT.reshape(n, 128, -1).transpose(1, 0, 2).reshape(128, -1)
    ).astype(NP16)


def kernel(x, attention_mask, Wq, Wk, Wv, Wo):
    x = np.asarray(x, dtype=np.float32)
    Wq = np.asarray(Wq, dtype=np.float32)
    Wk = np.asarray(Wk, dtype=np.float32)
    Wv = np.asarray(Wv, dtype=np.float32)
    Wo = np.asarray(Wo, dtype=np.float32)

    cosT, msinP = _rope_tables()
    xTb = [np.ascontiguousarray(x[b].T).astype(NP16) for b in range(B)]

    in_maps = []
    for c in range(NCORES):
        b = c // (NCORES // B)
        hg = c % (NCORES // B)
        rows = slice(hg * DPC, (hg + 1) * DPC)
        in_maps.append({
            "xT": xTb[b],
            "wqd": _pack_w(Wq[rows].T),
            "wkd": _pack_w(Wk[rows].T),
            "wvd": _pack_w(Wv[rows].T),
            "wod": _pack_w(Wo[:, rows].T),
            "cosd": cosT,
            "msinPd": msinP,
        })

    global _last_in_maps
    _last_in_maps = in_maps

    nc = _get_nc()
    res = run_bass_kernel_spmd(nc, in_maps, core_ids=list(range(NCORES)))
    global _LAST_RES
    _LAST_RES = res
    parts = [r["out"].astype(np.float32) for r in res.results]

    out = np.empty((B, N, D), dtype=np.float32)
    g = NCORES // B
    for b in range(B):
        out[b] = np.sum(np.stack(parts[b * g:(b + 1) * g]), axis=0)
    return out
